# revision 1
# baseline (speedup 1.0000x reference)
import numpy as np
import jax
import jax.numpy as jnp
from functools import partial

EPS = 1e-6
Bsz, L, H, P = 16, 4096, 128, 256
T = 128            # chunk length for the intra-chunk convolution
NC = L // T        # number of chunks
NCORES = 8
BPC = Bsz // NCORES  # sequences per core


def _host_tables(A_diag, G_diag, dt):
    """Per-mode parameter projection (f32, mirroring the reference) and
    matrix-power tables (f64 for stability), all O(P) / O(T*P) work."""
    f32 = np.float32
    dt_s = 1.0 / (1.0 + np.exp(-dt.astype(f32)))
    A = np.maximum(A_diag.astype(f32), f32(0.0))
    G = np.maximum(G_diag.astype(f32), f32(0.0))
    root = np.sqrt(f32(1.0) + dt_s * G)
    denom = np.maximum(dt_s * dt_s, f32(EPS))
    A_low = (f32(2.0) + dt_s * G - f32(2.0) * root) / denom
    A_high = (f32(2.0) + dt_s * G + f32(2.0) * root) / denom
    A = A_low + np.maximum(A - A_low, 0) - np.maximum(A - A_high, 0)
    S = f32(1.0) / (f32(1.0) + dt_s * G)

    # IMEX1 2x2 per-mode transition M = [[mA, mB], [mC, mD]], forcing scales c1, c2
    dt64, S64, A64 = dt_s.astype(np.float64), S.astype(np.float64), A.astype(np.float64)
    mA, mB = S64, -A64 * dt64 * S64
    mC, mD = dt64 * S64, 1.0 - A64 * dt64 * dt64 * S64
    c1, c2 = dt64 * S64, dt64 * dt64 * S64

    # powers M^d, d = 0..T (elementwise per mode)
    hA = np.zeros((T + 1, P)); hB = np.zeros((T + 1, P))
    hC = np.zeros((T + 1, P)); hD = np.zeros((T + 1, P))
    hA[0] = 1.0; hD[0] = 1.0
    for d in range(1, T + 1):
        hA[d] = mA * hA[d - 1] + mB * hC[d - 1]
        hB[d] = mA * hB[d - 1] + mB * hD[d - 1]
        hC[d] = mC * hA[d - 1] + mD * hC[d - 1]
        hD[d] = mC * hB[d - 1] + mD * hD[d - 1]

    # conv kernels: response of (z, x) at lag d to forcing (c1*Bu, c2*Bu)
    wZ = hA[:T] * c1 + hB[:T] * c2          # (T, P)
    wX = hC[:T] * c1 + hD[:T] * c2          # (T, P)

    # intra-chunk causal Toeplitz tensor: WXt[j, k, p] = wX[j-k, p] for k <= j
    idx = np.arange(T)
    dmat = idx[:, None] - idx[None, :]                       # (T, T)
    mask = dmat >= 0
    WXt = np.where(mask[:, :, None], wX[np.clip(dmat, 0, T - 1)], 0.0)  # (T,T,P)
    WZrow = wZ[::-1].copy()   # (T, P): weight for Bu[k] in z at chunk end
    WXrow = wX[::-1].copy()   # (T, P)

    # carry application: state at chunk start propagated j+1 steps, x-row
    hCj = hC[1:T + 1]         # (T, P)
    hDj = hD[1:T + 1]
    # chunk-to-chunk propagator M^T entries
    MT = np.stack([hA[T], hB[T], hC[T], hD[T]])  # (4, P)

    c = lambda a: jnp.asarray(a, jnp.float32)
    return dict(WXt=c(WXt), WZrow=c(WZrow), WXrow=c(WXrow),
                hCj=c(hCj), hDj=c(hDj), MT=c(MT))


def _core_fn(u, B0T, B1T, C0, C1, D, WXt, WZrow, WXrow, hCj, hDj, MT):
    # u: (BPC, L, H) on one core
    Bu_re = jnp.einsum('blh,hp->blp', u, B0T)        # (BPC, L, P)
    Bu_im = jnp.einsum('blh,hp->blp', u, B1T)
    Bu = jnp.stack([Bu_re, Bu_im], 0).reshape(2 * BPC, NC, T, P)

    # chunk-local end states (zero initial state within each chunk)
    z_loc = jnp.einsum('bckp,kp->bcp', Bu, WZrow)    # (2B, NC, P)
    x_loc = jnp.einsum('bckp,kp->bcp', Bu, WXrow)

    # carry scan across chunks: s_c = M^T s_{c-1} + s_loc[c]; emit s_{c-1}
    def step(s, sl):
        z, x = s
        zl, xl = sl
        zn = MT[0] * z + MT[1] * x + zl
        xn = MT[2] * z + MT[3] * x + xl
        return (zn, xn), (z, x)
    s0 = (jnp.zeros((2 * BPC, P)), jnp.zeros((2 * BPC, P)))
    _, (z_in, x_in) = jax.lax.scan(
        step, s0,
        (jnp.moveaxis(z_loc, 1, 0), jnp.moveaxis(x_loc, 1, 0)))
    z_in = jnp.moveaxis(z_in, 0, 1)                  # (2B, NC, P): state entering chunk
    x_in = jnp.moveaxis(x_in, 0, 1)

    # x states: intra-chunk causal conv + propagated carry
    x_intra = jnp.einsum('jkp,bckp->bcjp', WXt, Bu)  # (2B, NC, T, P)
    x_carry = hCj[None, None] * z_in[:, :, None] + hDj[None, None] * x_in[:, :, None]
    xs = (x_intra + x_carry).reshape(2 * BPC, L, P)

    xs_re, xs_im = xs[:BPC], xs[BPC:]
    ys = (jnp.einsum('blp,hp->blh', xs_re, C0)
          - jnp.einsum('blp,hp->blh', xs_im, C1)
          + D * u)
    return ys


def kernel(input_sequence, A_diag, G_diag, dt, B, C, D):
    tabs = _host_tables(np.asarray(A_diag), np.asarray(G_diag), np.asarray(dt))
    B = np.asarray(B); C = np.asarray(C)
    consts = dict(B0T=jnp.asarray(B[:, :, 0].T), B1T=jnp.asarray(B[:, :, 1].T),
                  C0=jnp.asarray(C[:, :, 0]), C1=jnp.asarray(C[:, :, 1]),
                  D=jnp.asarray(np.asarray(D)), **tabs)

    u = jnp.asarray(np.asarray(input_sequence)).reshape(NCORES, BPC, L, H)
    fn = jax.pmap(partial(_core_fn, **{}), in_axes=(0,) + (None,) * 11)
    out = fn(u, consts['B0T'], consts['B1T'], consts['C0'], consts['C1'],
             consts['D'], consts['WXt'], consts['WZrow'], consts['WXrow'],
             consts['hCj'], consts['hDj'], consts['MT'])
    return np.asarray(out).reshape(Bsz, L, H).astype(np.float32)


# revision 3
# speedup vs baseline: 1.2960x; 1.2960x over previous
import numpy as np
import jax
import jax.numpy as jnp

EPS = 1e-6
Bsz, L, H, P = 16, 4096, 128, 256
T = 128            # chunk length for the intra-chunk convolution
NC = L // T        # number of chunks
NCORES = 8
BPC = Bsz // NCORES  # sequences per core


def _host_tables(A_diag, G_diag, dt):
    """Per-mode parameter projection (f32, mirroring the reference) and
    matrix-power tables (f64 for stability), all O(P) / O(T*P) work."""
    f32 = np.float32
    dt_s = 1.0 / (1.0 + np.exp(-dt.astype(f32)))
    A = np.maximum(A_diag.astype(f32), f32(0.0))
    G = np.maximum(G_diag.astype(f32), f32(0.0))
    root = np.sqrt(f32(1.0) + dt_s * G)
    denom = np.maximum(dt_s * dt_s, f32(EPS))
    A_low = (f32(2.0) + dt_s * G - f32(2.0) * root) / denom
    A_high = (f32(2.0) + dt_s * G + f32(2.0) * root) / denom
    A = A_low + np.maximum(A - A_low, 0) - np.maximum(A - A_high, 0)
    S = f32(1.0) / (f32(1.0) + dt_s * G)

    # IMEX1 2x2 per-mode transition M = [[mA, mB], [mC, mD]], forcing scales c1, c2
    dt64, S64, A64 = dt_s.astype(np.float64), S.astype(np.float64), A.astype(np.float64)
    mA, mB = S64, -A64 * dt64 * S64
    mC, mD = dt64 * S64, 1.0 - A64 * dt64 * dt64 * S64
    c1, c2 = dt64 * S64, dt64 * dt64 * S64

    # powers M^d, d = 0..T (elementwise per mode)
    hA = np.zeros((T + 1, P)); hB = np.zeros((T + 1, P))
    hC = np.zeros((T + 1, P)); hD = np.zeros((T + 1, P))
    hA[0] = 1.0; hD[0] = 1.0
    for d in range(1, T + 1):
        hA[d] = mA * hA[d - 1] + mB * hC[d - 1]
        hB[d] = mA * hB[d - 1] + mB * hD[d - 1]
        hC[d] = mC * hA[d - 1] + mD * hC[d - 1]
        hD[d] = mC * hB[d - 1] + mD * hD[d - 1]

    # conv kernels: response of (z, x) at lag d to forcing (c1*Bu, c2*Bu)
    wZ = hA[:T] * c1 + hB[:T] * c2          # (T, P)
    wX = hC[:T] * c1 + hD[:T] * c2          # (T, P)

    # intra-chunk causal Toeplitz tensor: WXt[j, k, p] = wX[j-k, p] for k <= j
    idx = np.arange(T)
    dmat = idx[:, None] - idx[None, :]                       # (T, T)
    mask = dmat >= 0
    WXt = np.where(mask[:, :, None], wX[np.clip(dmat, 0, T - 1)], 0.0)  # (T,T,P)
    WZrow = wZ[::-1].copy()   # (T, P): weight for Bu[k] in z at chunk end
    WXrow = wX[::-1].copy()   # (T, P)

    # carry application: state at chunk start propagated j+1 steps, x-row
    hCj = hC[1:T + 1]         # (T, P)
    hDj = hD[1:T + 1]
    # chunk-to-chunk propagator M^T entries
    MT = np.stack([hA[T], hB[T], hC[T], hD[T]])  # (4, P)

    c = lambda a: jnp.asarray(a, jnp.float32)
    return dict(WXt=c(WXt), WZrow=c(WZrow), WXrow=c(WXrow),
                hCj=c(hCj), hDj=c(hDj), MT=c(MT))


def _core_fn(u, B0T, B1T, C0, C1, D, WXt, WZrow, WXrow, hCj, hDj, MT):
    # u: (BPC, L, H) on one core
    Bu_re = jnp.einsum('blh,hp->blp', u, B0T)        # (BPC, L, P)
    Bu_im = jnp.einsum('blh,hp->blp', u, B1T)
    Bu = jnp.stack([Bu_re, Bu_im], 0).reshape(2 * BPC, NC, T, P)

    # chunk-local end states (zero initial state within each chunk)
    z_loc = jnp.einsum('bckp,kp->bcp', Bu, WZrow)    # (2B, NC, P)
    x_loc = jnp.einsum('bckp,kp->bcp', Bu, WXrow)

    # carry scan across chunks: s_c = M^T s_{c-1} + s_loc[c]; emit s_{c-1}
    def step(s, sl):
        z, x = s
        zl, xl = sl
        zn = MT[0] * z + MT[1] * x + zl
        xn = MT[2] * z + MT[3] * x + xl
        return (zn, xn), (z, x)
    s0 = (jnp.zeros((2 * BPC, P)), jnp.zeros((2 * BPC, P)))
    _, (z_in, x_in) = jax.lax.scan(
        step, s0,
        (jnp.moveaxis(z_loc, 1, 0), jnp.moveaxis(x_loc, 1, 0)))
    z_in = jnp.moveaxis(z_in, 0, 1)                  # (2B, NC, P): state entering chunk
    x_in = jnp.moveaxis(x_in, 0, 1)

    # x states: intra-chunk causal conv + propagated carry
    x_intra = jnp.einsum('jkp,bckp->bcjp', WXt, Bu)  # (2B, NC, T, P)
    x_carry = hCj[None, None] * z_in[:, :, None] + hDj[None, None] * x_in[:, :, None]
    xs = (x_intra + x_carry).reshape(2 * BPC, L, P)

    xs_re, xs_im = xs[:BPC], xs[BPC:]
    ys = (jnp.einsum('blp,hp->blh', xs_re, C0)
          - jnp.einsum('blp,hp->blh', xs_im, C1)
          + D * u)
    return ys


def kernel(input_sequence, A_diag, G_diag, dt, B, C, D):
    tabs = _host_tables(np.asarray(A_diag), np.asarray(G_diag), np.asarray(dt))
    B = np.asarray(B); C = np.asarray(C)
    consts = dict(B0T=jnp.asarray(B[:, :, 0].T), B1T=jnp.asarray(B[:, :, 1].T),
                  C0=jnp.asarray(C[:, :, 0]), C1=jnp.asarray(C[:, :, 1]),
                  D=jnp.asarray(np.asarray(D)), **tabs)

    u = jnp.asarray(np.asarray(input_sequence)).reshape(NCORES, BPC, L, H)
    fn = jax.pmap(_core_fn, in_axes=(0,) + (None,) * 11)
    out = fn(u, consts['B0T'], consts['B1T'], consts['C0'], consts['C1'],
             consts['D'], consts['WXt'], consts['WZrow'], consts['WXrow'],
             consts['hCj'], consts['hDj'], consts['MT'])
    return np.asarray(out).reshape(Bsz, L, H).astype(np.float32)
